# revision 4
# baseline (speedup 1.0000x reference)
"""Trainium2 Bass kernel for top-2 MoE (nn_MoE_2113123910117).

Strategy (expert-parallel, per sharding hint):
  - Host: router logits -> softmax -> top-2 -> normalized combine weights;
    dispatch tokens to 8 expert shards (one expert per NeuronCore).
  - Device (per core): SwiGLU expert FFN over its gathered tokens,
    y = diag(scale) @ ((silu(x Wg^T) * (x Wu^T)) Wd^T), fp16 matmul
    operands with fp32 PSUM accumulation.
  - Host: scatter-add per-expert outputs back into the [B,T,D] output.

Schedule notes:
  - Gate/up weights are host-packed per f-tile (wgu[f] = [128, 2*D] holding
    the 8 gate d-blocks then the 8 up d-blocks) so the PE can start the
    first gate chain after ~1.3MB of DMA instead of waiting for the full
    4MB wg stream.
  - Token tiles are balanced multiples of 128 with max 512 (PSUM bank cap)
    and min 384 so LDWEIGHTS stays hidden under the matmul stream (a
    trailing 128-token tile runs at ~half PE efficiency).

Self-contained: hardcodes all shapes from the problem spec.
"""

import os
import numpy as np

D = 1024
FF = 2048
E = 8
TOPK = 2
NCORES = 8
ND = D // 128    # 8 contraction chunks
NF = FF // 128   # 16 ff chunks
MIN_CAP = 2176   # >= max expert load for the spec'd input, multiple of 128

# matmul operand dtype on device ("float16", "bfloat16")
MM_DTYPE = os.environ.get("MOE_MM_DTYPE", "float16")

# test-only knobs / results (harness never touches these)
LAST_RESULTS = None
_NC_CACHE = {}


def split_multi_waits(nc, mybir_mod):
    """This walrus build rejects any instruction carrying more than one
    sync wait ("Too many sync wait commands"). Hoist extra waits onto
    single-wait NOPs inserted just before the instruction on the same
    engine — semantically identical since engines execute in order."""
    n_split = 0
    for f in nc.m.functions:
        for blk in f.blocks:
            insts = blk.instructions
            newl = []
            changed = False
            for inst in insts:
                si = inst.sync_info
                if si is not None and len(si.on_wait) > 1:
                    waits = list(si.on_wait)
                    del si.on_wait[1:]
                    for j, w in enumerate(waits[1:]):
                        nop = mybir_mod.InstNoOp(
                            name=f"{inst.name}_w{j}",
                            engine=inst.engine,
                            ins=[],
                            outs=[],
                        )
                        nop.sync_info = mybir_mod.SyncInfo(on_wait=[w], on_update=[])
                        newl.append(nop)
                        n_split += 1
                    changed = True
                newl.append(inst)
            if changed:
                insts[:] = newl
    return n_split


def _token_tiles(cap):
    """Balanced token tiles: multiples of 128, each <= 512, sized as evenly
    as possible, smallest first (cheapest x-DMA before first matmul)."""
    ngroups = cap // 128
    ntiles = -(-ngroups // 4)
    base, extra = divmod(ngroups, ntiles)
    sizes = [base + (1 if i >= ntiles - extra else 0) for i in range(ntiles)]
    tiles = []
    off = 0
    for g in sizes:
        tiles.append((off, g * 128))
        off += g * 128
    return tiles


def build_nc(cap, repeat=1):
    """Build the per-core Bass program: SwiGLU FFN for one expert over
    `cap` (padded) tokens. Same NEFF on all 8 cores (SPMD).

    repeat>1 wraps the whole body (including weight loads) in a hardware
    loop — used only for benchmarking (dispatch overhead amortization)."""
    import contextlib

    import concourse.bass as bass
    import concourse.mybir as mybir
    import concourse.tile as tile

    dt = mybir.dt
    f32 = dt.float32
    mmdt = getattr(dt, MM_DTYPE)
    AF = mybir.ActivationFunctionType
    NG = cap // 128  # token 128-groups

    nc = bass.Bass()
    xt = nc.dram_tensor("xt", [D, cap], mmdt, kind="ExternalInput")
    wgu = nc.dram_tensor("wgu", [NF * 128, 2 * D], mmdt, kind="ExternalInput")
    wd = nc.dram_tensor("wd", [FF, D], mmdt, kind="ExternalInput")
    sc = nc.dram_tensor("sc", [128, NG], f32, kind="ExternalInput")
    y = nc.dram_tensor("y", [cap, D], f32, kind="ExternalOutput")

    with tile.TileContext(nc) as tc:
        with (
            tc.tile_pool(name="wpool", bufs=1) as wpool,
            tc.tile_pool(name="xpool", bufs=2) as xpool,
            tc.tile_pool(name="hpool", bufs=2) as hpool,
            tc.tile_pool(name="gpool", bufs=3) as gpool,
            tc.tile_pool(name="ypool", bufs=3) as ypool,
            tc.tile_pool(name="pg", bufs=2, space="PSUM") as pgpool,
            tc.tile_pool(name="pu", bufs=2, space="PSUM") as pupool,
            tc.tile_pool(name="po", bufs=4, space="PSUM") as popool,
            (
                tc.For_i(0, repeat, 1, hint_engines=(mybir.EngineType.PE,))
                if repeat > 1
                else contextlib.nullcontext()
            ),
        ):
            tiles = _token_tiles(cap)
            # --- DMA order is the startup critical path ---
            # wgu[0] + x tile 0 unblock the first gate chain (~2.3MB);
            # wd can wait until tile 0's gate/up phase ends (~40us in).
            wgu_sb = [None] * NF
            wgu_sb[0] = wpool.tile([128, 2 * D], mmdt, tag="wgu0", name="wgu_sb0")
            nc.sync.dma_start(wgu_sb[0][:], wgu[0:128, :])

            off0, tt0 = tiles[0]
            xt0 = []
            for d in range(ND):
                t = xpool.tile([128, tt0], mmdt, tag=f"xt{d}")
                nc.sync.dma_start(t[:], xt[d * 128 : (d + 1) * 128, off0 : off0 + tt0])
                xt0.append(t)
            for f in range(1, 14):
                wgu_sb[f] = wpool.tile([128, 2 * D], mmdt, tag=f"wgu{f}",
                                       name=f"wgu_sb{f}")
                nc.sync.dma_start(wgu_sb[f][:], wgu[f * 128 : (f + 1) * 128, :])
            s_sb = wpool.tile([128, NG], f32, tag="s")
            nc.sync.dma_start(s_sb[:], sc[:])
            wd_sb = []
            for f in range(NF):
                t = wpool.tile([128, D], mmdt, tag=f"wd{f}")
                nc.sync.dma_start(t[:], wd[f * 128 : (f + 1) * 128, :])
                wd_sb.append(t)
            for f in range(14, NF):
                wgu_sb[f] = wpool.tile([128, 2 * D], mmdt, tag=f"wgu{f}",
                                       name=f"wgu_sb{f}")
                nc.sync.dma_start(wgu_sb[f][:], wgu[f * 128 : (f + 1) * 128, :])

            for off, tt in tiles:
                # x^T tile: [d, tokens]
                if off == off0:
                    xt_t = xt0
                else:
                    xt_t = []
                    for d in range(ND):
                        t = xpool.tile([128, tt], mmdt, tag=f"xt{d}")
                        nc.sync.dma_start(
                            t[:], xt[d * 128 : (d + 1) * 128, off : off + tt]
                        )
                        xt_t.append(t)
                # gate/up + SwiGLU -> h^T [f, tokens]
                ht_t = []
                for f in range(NF):
                    pg = pgpool.tile([128, tt], f32, tag="pg")
                    pu = pupool.tile([128, tt], f32, tag="pu")
                    for d in range(ND):
                        nc.tensor.matmul(
                            pg[:],
                            wgu_sb[f][:, d * 128 : (d + 1) * 128],
                            xt_t[d][:],
                            start=(d == 0),
                            stop=(d == ND - 1),
                        )
                    for d in range(ND):
                        nc.tensor.matmul(
                            pu[:],
                            wgu_sb[f][:, D + d * 128 : D + (d + 1) * 128],
                            xt_t[d][:],
                            start=(d == 0),
                            stop=(d == ND - 1),
                        )
                    sg = gpool.tile([128, tt], mmdt, tag="sg")
                    nc.scalar.activation(sg[:], pg[:], AF.Silu)
                    ht = hpool.tile([128, tt], mmdt, tag=f"ht{f}")
                    nc.vector.tensor_mul(ht[:], sg[:], pu[:])
                    ht_t.append(ht)
                # down projection, scaled by combine weight per token
                for k in range(tt // 128):
                    g = off // 128 + k
                    po_h = []
                    for dh in range(2):
                        po = popool.tile([128, 512], f32, tag="po", name=f"po_{off}_{k}_{dh}")
                        po_h.append(po)
                    for f in range(NF):
                        lhs = ht_t[f][:, k * 128 : (k + 1) * 128]
                        for dh in range(2):
                            nc.tensor.matmul(
                                po_h[dh][:],
                                lhs,
                                wd_sb[f][:, dh * 512 : (dh + 1) * 512],
                                start=(f == 0),
                                stop=(f == NF - 1),
                            )
                    for dh in range(2):
                        yt = ypool.tile([128, 512], f32, tag="yt")
                        nc.scalar.activation(
                            yt[:], po_h[dh][:], AF.Copy, scale=s_sb[:, g : g + 1]
                        )
                        nc.sync.dma_start(
                            y[off + k * 128 : off + (k + 1) * 128,
                              dh * 512 : (dh + 1) * 512],
                            yt[:],
                        )
    split_multi_waits(nc, mybir)
    return nc


def _get_nc(cap):
    key = (cap, MM_DTYPE)
    if key not in _NC_CACHE:
        _NC_CACHE[key] = build_nc(cap)
    return _NC_CACHE[key]


def _route(xf, Wr):
    """fp32 softmax + top-2 + normalized combine weights, matching the
    jax reference (ties broken toward lower expert index)."""
    logits = xf @ Wr.astype(np.float32).T
    m = logits.max(-1, keepdims=True)
    ex = np.exp(logits - m)
    p = ex / ex.sum(-1, keepdims=True)
    top2 = np.argsort(-p, axis=-1, kind="stable")[:, :TOPK]
    n = xf.shape[0]
    p1 = p[np.arange(n), top2[:, 0]]
    p2 = p[np.arange(n), top2[:, 1]]
    denom = (p1 + p2) + np.float32(1e-8)
    return top2, p1 / denom, p2 / denom


def _pack_wgu(Wg_e, Wu_e, mmnp):
    """Host-pack gate/up weights per f-tile: out[f*128+p, d*128+c] =
    WgT[d*128+p, f*128+c] for the gate half, same for up at col offset D."""
    out = np.empty((NF * 128, 2 * D), dtype=mmnp)
    for half, W in ((0, Wg_e), (1, Wu_e)):
        WT = np.ascontiguousarray(W.T).astype(mmnp)          # [D, FF]
        A = WT.reshape(ND, 128, NF, 128)                     # [d, p, f, c]
        out[:, half * D : (half + 1) * D] = (
            A.transpose(2, 1, 0, 3).reshape(NF * 128, D)
        )
    return out


def make_in_maps(x, Wr, Wg, Wu, Wd):
    """Route on host, build per-core device input maps. Returns
    (in_maps, idxs, cap, n_tok)."""
    xf = x.reshape(-1, D).astype(np.float32, copy=False)
    top2, s1, s2 = _route(xf, Wr)

    mmnp = np.dtype(np.float16 if MM_DTYPE == "float16" else np.float32)
    if MM_DTYPE == "bfloat16":
        import ml_dtypes

        mmnp = np.dtype(ml_dtypes.bfloat16)

    xf_mm = xf.astype(mmnp)

    idxs = []
    counts = []
    for e in range(E):
        idx = np.nonzero((top2[:, 0] == e) | (top2[:, 1] == e))[0]
        idxs.append(idx)
        counts.append(len(idx))
    cap = max(MIN_CAP, -(-max(counts) // 128) * 128)

    in_maps = []
    for e in range(E):
        idx = idxs[e]
        n_e = len(idx)
        xt = np.zeros((D, cap), dtype=mmnp)
        xt[:, :n_e] = xf_mm[idx].T
        scv = np.zeros(cap, dtype=np.float32)
        scv[:n_e] = np.where(top2[idx, 0] == e, s1[idx], s2[idx])
        sc2d = np.ascontiguousarray(scv.reshape(cap // 128, 128).T)
        in_maps.append(
            {
                "xt": xt,
                "wgu": _pack_wgu(Wg[e], Wu[e], mmnp),
                "wd": np.ascontiguousarray(Wd[e].T).astype(mmnp),
                "sc": sc2d,
            }
        )
    return in_maps, idxs, cap, xf.shape[0]


def kernel(**inputs):
    global LAST_RESULTS
    from concourse.bass_utils import run_bass_kernel_spmd

    x = np.asarray(inputs["x"])
    B, T, _ = x.shape
    in_maps, idxs, cap, n_tok = make_in_maps(
        x, np.asarray(inputs["Wr"]), np.asarray(inputs["Wg"]),
        np.asarray(inputs["Wu"]), np.asarray(inputs["Wd"]),
    )

    nc = _get_nc(cap)
    res = run_bass_kernel_spmd(nc, in_maps, list(range(NCORES)))
    LAST_RESULTS = res

    out = np.zeros((n_tok, D), dtype=np.float32)
    for e in range(E):
        idx = idxs[e]
        out[idx] += res.results[e]["y"][: len(idx)]
    return out.reshape(B, T, D).astype(x.dtype, copy=False)


# revision 14
# speedup vs baseline: 1.0150x; 1.0150x over previous
"""Trainium2 Bass kernel for top-2 MoE (nn_MoE_2113123910117).

Strategy (expert-parallel with FF-split load balancing):
  - Host: router logits -> softmax -> top-2 -> normalized combine weights.
  - Work is split into 16 shards: (expert e, ff-half h), each covering the
    expert's routed tokens x one half of the FF dimension (SwiGLU is
    elementwise in f, and the down projection is linear in f, so ff-halves
    produce independent partial outputs that the host scatter-ADD combines).
  - Shards are paired onto 8 cores (largest with smallest), giving every
    core the same two-segment program shape (G1-group + G2-group segments);
    token-group imbalance drops from 17 full-FF groups to 16.5 equivalents.
  - Device (per core, per segment): y_partial = diag(s) @
    ((silu(x WgT_half) * (x WuT_half)) WdT_half), fp16 matmuls with fp32
    PSUM accumulation.
  - Host: scatter-add per-shard partial outputs into the [B,T,D] output.

Schedule notes:
  - Gate/up weights are host-packed per f-tile (row block f of wgu =
    [128, 2*D] holding 8 gate d-blocks then 8 up d-blocks) so the PE can
    start the first gate chain after ~1.3MB of DMA instead of the full
    gate-weight stream.
  - DMA order is the startup critical path: wguA[0], x tile 0, rest of
    wguA, sc, wdA, then segment B's weights; x tiles prefetch one tile
    ahead through a double-buffered pool (crossing the segment boundary).
  - Token tiles are balanced multiples of 128 with max 512 (PSUM bank cap)
    and min 384 so LDWEIGHTS stays hidden under the matmul stream.

Self-contained: hardcodes all shapes from the problem spec.
"""

import os
import numpy as np

D = 1024
FF = 2048
E = 8
TOPK = 2
NCORES = 8
ND = D // 128     # 8 contraction chunks (gate/up)
NFH = FF // 2 // 128  # 8 ff chunks per half-shard
MIN_G = (17, 16)  # segment group counts for the spec'd input

# matmul operand dtype on device ("float16", "bfloat16")
MM_DTYPE = os.environ.get("MOE_MM_DTYPE", "float16")

# test-only knobs / results (harness never touches these)
LAST_RESULTS = None
_NC_CACHE = {}


def split_multi_waits(nc, mybir_mod):
    """This walrus build rejects any instruction carrying more than one
    sync wait ("Too many sync wait commands"). Hoist extra waits onto
    single-wait NOPs inserted just before the instruction on the same
    engine — semantically identical since engines execute in order."""
    n_split = 0
    for f in nc.m.functions:
        for blk in f.blocks:
            insts = blk.instructions
            newl = []
            changed = False
            for inst in insts:
                si = inst.sync_info
                if si is not None and len(si.on_wait) > 1:
                    waits = list(si.on_wait)
                    del si.on_wait[1:]
                    for j, w in enumerate(waits[1:]):
                        nop = mybir_mod.InstNoOp(
                            name=f"{inst.name}_w{j}",
                            engine=inst.engine,
                            ins=[],
                            outs=[],
                        )
                        nop.sync_info = mybir_mod.SyncInfo(on_wait=[w], on_update=[])
                        newl.append(nop)
                        n_split += 1
                    changed = True
                newl.append(inst)
            if changed:
                insts[:] = newl
    return n_split


def _token_tiles(ngroups):
    """Balanced token tiles (in groups of 128): each tile <= 4 groups,
    sized as evenly as possible, smallest first."""
    ntiles = -(-ngroups // 4)
    base, extra = divmod(ngroups, ntiles)
    sizes = [base + (1 if i >= ntiles - extra else 0) for i in range(ntiles)]
    tiles = []
    off = 0
    for g in sizes:
        tiles.append((off, g * 128))
        off += g * 128
    return tiles


def build_nc(segs, repeat=1):
    """Per-core Bass program: two half-FF FFN segments with group counts
    ``segs = (G1, G2)`` over zero-padded token buffers. Same NEFF on all
    8 cores (SPMD).

    repeat>1 wraps the body in a hardware loop (benchmark-only)."""
    import contextlib

    import concourse.bass as bass
    import concourse.mybir as mybir
    import concourse.tile as tile

    dt = mybir.dt
    f32 = dt.float32
    mmdt = getattr(dt, MM_DTYPE)
    AF = mybir.ActivationFunctionType
    G1, G2 = segs
    NG = G1 + G2                     # total token groups across segments
    CAP = NG * 128

    nc = bass.Bass()
    xt = nc.dram_tensor("xt", [D, CAP], mmdt, kind="ExternalInput")
    # rows [s*1024 + f*128 ...]: f-tile f of segment s (gate cols 0:D, up D:2D)
    wgu = nc.dram_tensor("wgu", [2 * NFH * 128, 2 * D], mmdt, kind="ExternalInput")
    # rows [s*1024 ...]: segment s's half of Wd^T ([FF/2, D])
    wd = nc.dram_tensor("wd", [2 * NFH * 128, D], mmdt, kind="ExternalInput")
    sc = nc.dram_tensor("sc", [128, NG], f32, kind="ExternalInput")
    y = nc.dram_tensor("y", [CAP, D], f32, kind="ExternalOutput")

    # flat tile list across segments: (seg, group_offset_global, ntok)
    all_tiles = []
    for s, G in enumerate(segs):
        goff = 0 if s == 0 else G1
        for off, tt in _token_tiles(G):
            all_tiles.append((s, goff * 128 + off, tt))

    with tile.TileContext(nc) as tc:
        with (
            tc.tile_pool(name="wpool", bufs=1) as wpool,
            tc.tile_pool(name="xpool", bufs=3) as xpool,
            tc.tile_pool(name="hpool", bufs=2) as hpool,
            tc.tile_pool(name="gpool", bufs=3) as gpool,
            tc.tile_pool(name="ypool", bufs=3) as ypool,
            tc.tile_pool(name="pg", bufs=2, space="PSUM") as pgpool,
            tc.tile_pool(name="pu", bufs=2, space="PSUM") as pupool,
            tc.tile_pool(name="po", bufs=4, space="PSUM") as popool,
            (
                tc.For_i(0, repeat, 1, hint_engines=(mybir.EngineType.PE,))
                if repeat > 1
                else contextlib.nullcontext()
            ),
        ):
            # --- DMA order is the startup critical path ---
            wgu_sb = [[None] * NFH for _ in range(2)]
            wd_sb = [[None] * NFH for _ in range(2)]

            def load_wgu(s, f, split=False):
                t = wpool.tile([128, 2 * D], mmdt, tag=f"wgu{s}_{f}",
                               name=f"wgu_sb{s}_{f}")
                r = s * NFH * 128 + f * 128
                if split:
                    # gate half first: unblocks the first gate chain sooner
                    nc.sync.dma_start(t[:, 0:D], wgu[r : r + 128, 0:D])
                    nc.sync.dma_start(t[:, D : 2 * D], wgu[r : r + 128, D : 2 * D])
                else:
                    nc.sync.dma_start(t[:], wgu[r : r + 128, :])
                wgu_sb[s][f] = t

            def load_wd(s, f):
                t = wpool.tile([128, D], mmdt, tag=f"wd{s}_{f}",
                               name=f"wd_sb{s}_{f}")
                r = s * NFH * 128 + f * 128
                nc.sync.dma_start(t[:], wd[r : r + 128, :])
                wd_sb[s][f] = t

            def load_x(tile_idx):
                _, toff, tt = all_tiles[tile_idx]
                xs = []
                for d in range(ND):
                    t = xpool.tile([128, tt], mmdt, tag=f"xt{d}",
                                   name=f"x_{tile_idx}_{d}")
                    nc.sync.dma_start(t[:], xt[d * 128 : (d + 1) * 128,
                                               toff : toff + tt])
                    xs.append(t)
                return xs

            load_wgu(0, 0)
            x_sb = {0: load_x(0)}
            for f in range(1, NFH):
                load_wgu(0, f)
            s_sb = wpool.tile([128, NG], f32, tag="s")
            nc.sync.dma_start(s_sb[:], sc[:])
            x_sb[1] = load_x(1)
            for f in range(NFH):
                load_wd(0, f)
            x_sb[2] = load_x(2)
            for f in range(NFH):
                load_wgu(1, f)
            for f in range(NFH):
                load_wd(1, f)

            for ti, (s, toff, tt) in enumerate(all_tiles):
                xt_t = x_sb.pop(ti)
                if ti + 3 < len(all_tiles):
                    x_sb[ti + 3] = load_x(ti + 3)  # triple-buffered prefetch
                # gate/up + SwiGLU -> h^T [f, tokens]
                ht_t = []
                for f in range(NFH):
                    pg = pgpool.tile([128, tt], f32, tag="pg")
                    pu = pupool.tile([128, tt], f32, tag="pu")
                    for d in range(ND):
                        nc.tensor.matmul(
                            pg[:],
                            wgu_sb[s][f][:, d * 128 : (d + 1) * 128],
                            xt_t[d][:],
                            start=(d == 0),
                            stop=(d == ND - 1),
                        )
                    for d in range(ND):
                        nc.tensor.matmul(
                            pu[:],
                            wgu_sb[s][f][:, D + d * 128 : D + (d + 1) * 128],
                            xt_t[d][:],
                            start=(d == 0),
                            stop=(d == ND - 1),
                        )
                    sg = gpool.tile([128, tt], mmdt, tag="sg")
                    nc.scalar.activation(sg[:], pg[:], AF.Silu)
                    ht = hpool.tile([128, tt], mmdt, tag=f"ht{f}")
                    nc.vector.tensor_mul(ht[:], sg[:], pu[:])
                    ht_t.append(ht)
                # down projection (partial: half the ff contraction),
                # scaled by combine weight per token
                for k in range(tt // 128):
                    g = toff // 128 + k
                    po_h = []
                    for dh in range(2):
                        po = popool.tile([128, 512], f32, tag="po",
                                         name=f"po_{toff}_{k}_{dh}")
                        po_h.append(po)
                    for f in range(NFH):
                        lhs = ht_t[f][:, k * 128 : (k + 1) * 128]
                        for dh in range(2):
                            nc.tensor.matmul(
                                po_h[dh][:],
                                lhs,
                                wd_sb[s][f][:, dh * 512 : (dh + 1) * 512],
                                start=(f == 0),
                                stop=(f == NFH - 1),
                            )
                    for dh in range(2):
                        yt = ypool.tile([128, 512], f32, tag="yt")
                        nc.scalar.activation(
                            yt[:], po_h[dh][:], AF.Copy, scale=s_sb[:, g : g + 1]
                        )
                        nc.sync.dma_start(
                            y[toff + k * 128 : toff + (k + 1) * 128,
                              dh * 512 : (dh + 1) * 512],
                            yt[:],
                        )
    split_multi_waits(nc, mybir)
    return nc


def _get_nc(segs):
    key = (segs, MM_DTYPE)
    if key not in _NC_CACHE:
        _NC_CACHE[key] = build_nc(segs)
    return _NC_CACHE[key]


def _route(xf, Wr):
    """fp32 softmax + top-2 + normalized combine weights, matching the
    jax reference (ties broken toward lower expert index)."""
    logits = xf @ Wr.astype(np.float32).T
    m = logits.max(-1, keepdims=True)
    ex = np.exp(logits - m)
    p = ex / ex.sum(-1, keepdims=True)
    top2 = np.argsort(-p, axis=-1, kind="stable")[:, :TOPK]
    n = xf.shape[0]
    p1 = p[np.arange(n), top2[:, 0]]
    p2 = p[np.arange(n), top2[:, 1]]
    denom = (p1 + p2) + np.float32(1e-8)
    return top2, p1 / denom, p2 / denom


def _pack_wgu_half(Wg_half, Wu_half, mmnp):
    """Pack one ff-half of gate/up weights per f-tile:
    out[f*128+p, d*128+c] = W^T[d*128+p, f*128+c], gate in cols [0,D),
    up in cols [D,2D). W*_half: [FF/2, D]."""
    nf = Wg_half.shape[0] // 128
    out = np.empty((nf * 128, 2 * D), dtype=mmnp)
    for half, W in ((0, Wg_half), (1, Wu_half)):
        WT = np.ascontiguousarray(W.T).astype(mmnp)          # [D, FF/2]
        A = WT.reshape(ND, 128, nf, 128)                     # [d, p, f, c]
        out[:, half * D : (half + 1) * D] = (
            A.transpose(2, 1, 0, 3).reshape(nf * 128, D)
        )
    return out


def make_in_maps(x, Wr, Wg, Wu, Wd):
    """Route on host, pair (expert, ff-half) shards onto cores, build
    per-core device input maps. Returns (in_maps, assignments, segs, n_tok)
    where assignments[c] = [(e, h, n_e, idx_e), (e, h, n_e, idx_e)]."""
    xf = x.reshape(-1, D).astype(np.float32, copy=False)
    top2, s1, s2 = _route(xf, Wr)

    mmnp = np.dtype(np.float16 if MM_DTYPE == "float16" else np.float32)
    if MM_DTYPE == "bfloat16":
        import ml_dtypes

        mmnp = np.dtype(ml_dtypes.bfloat16)

    xf_mm = xf.astype(mmnp)

    idxs, scs, groups = [], [], []
    for e in range(E):
        idx = np.nonzero((top2[:, 0] == e) | (top2[:, 1] == e))[0]
        idxs.append(idx)
        scs.append(np.where(top2[idx, 0] == e, s1[idx], s2[idx]).astype(np.float32))
        groups.append(max(1, -(-len(idx) // 128)))

    # shards (e, h) sorted by size desc; pair i-th largest with i-th smallest
    shards = sorted(
        [(e, h) for e in range(E) for h in range(2)],
        key=lambda eh: (-groups[eh[0]], eh[0], eh[1]),
    )
    assignments = []
    for c in range(NCORES):
        assignments.append([shards[c], shards[2 * NCORES - 1 - c]])
    G1 = max(groups[a[0][0]] for a in assignments)
    G2 = max(groups[a[1][0]] for a in assignments)
    G1, G2 = max(G1, MIN_G[0]), max(G2, MIN_G[1])
    segs = (G1, G2)
    CAP = (G1 + G2) * 128

    in_maps = []
    asg_meta = []
    for c in range(NCORES):
        xt = np.zeros((D, CAP), dtype=mmnp)
        wgu = np.empty((2 * NFH * 128, 2 * D), dtype=mmnp)
        wd = np.empty((2 * NFH * 128, D), dtype=mmnp)
        scv = np.zeros(CAP, dtype=np.float32)
        meta = []
        for s, (e, h) in enumerate(assignments[c]):
            idx = idxs[e]
            n_e = len(idx)
            coff = 0 if s == 0 else G1 * 128
            xt[:, coff : coff + n_e] = xf_mm[idx].T
            scv[coff : coff + n_e] = scs[e]
            rows = slice(s * NFH * 128, (s + 1) * NFH * 128)
            frows = slice(h * (FF // 2), (h + 1) * (FF // 2))
            wgu[rows] = _pack_wgu_half(Wg[e][frows], Wu[e][frows], mmnp)
            wd[rows] = np.ascontiguousarray(Wd[e].T[frows]).astype(mmnp)
            meta.append((e, h, n_e))
        sc2d = np.ascontiguousarray(scv.reshape(-1, 128).T)
        in_maps.append({"xt": xt, "wgu": wgu, "wd": wd, "sc": sc2d})
        asg_meta.append(meta)
    return in_maps, asg_meta, segs, xf.shape[0], idxs


def kernel(**inputs):
    global LAST_RESULTS
    from concourse.bass_utils import run_bass_kernel_spmd

    x = np.asarray(inputs["x"])
    B, T, _ = x.shape
    in_maps, asg, segs, n_tok, idxs = make_in_maps(
        x, np.asarray(inputs["Wr"]), np.asarray(inputs["Wg"]),
        np.asarray(inputs["Wu"]), np.asarray(inputs["Wd"]),
    )

    nc = _get_nc(segs)
    res = run_bass_kernel_spmd(nc, in_maps, list(range(NCORES)))
    LAST_RESULTS = res

    out = np.zeros((n_tok, D), dtype=np.float32)
    G1 = segs[0]
    for c in range(NCORES):
        yc = res.results[c]["y"]
        for s, (e, h, n_e) in enumerate(asg[c]):
            coff = 0 if s == 0 else G1 * 128
            out[idxs[e]] += yc[coff : coff + n_e]
    return out.reshape(B, T, D).astype(x.dtype, copy=False)
